# revision 52
# baseline (speedup 1.0000x reference)
"""MiniFastSpeech Trainium2 kernel (v2: weight-stationary bf16 LSTM).

Strategy:
- Host (numpy): embed lookup, duration predictor, cumsum, searchsorted
  length-regulator expansion -> exp [B, L, E]; pad to L_PAD = 16*CHUNK.
- Device (8 cores, SPMD): bidirectional LSTM via sequence-chunked
  parallelism with zero-state warmup (state sensitivity decays ~0.75/step;
  W=11 measured 7.8e-3 output rel err on HW, budget 2e-2).
- Core j runs two lockstep pair-chains (fwd chunks (2j,2j+1), bwd chunks
  (15-2j,14-2j) over the reversed sequence), 128 lanes = 64 batch x 2
  chunks each.
- Weight-stationary layout: gates live in two PSUM tiles per chain
  (bank A = [f0 f1 i0 i1], bank B = [o0 o1 g0 g1]; each [128 part =
  gate-dim-chunk, 4 m-chunks x 128 lanes]) so sigmoid(A) only waits on
  A's 8 recurrent matmuls. Recurrent matmuls: stationary = whh tile
  [128 hid-k, 128 gate-m] bf16, moving = h tile [128 hid-k, 128 lanes]
  bf16 (1 cyc/row at any size), k-major order so k0 mms start right
  after the h0 half-write. h is written ONCE per step as bf16 directly
  into the per-k-chunk X accumulators (which double as the h stream for
  the next step's moving operand and phase-2's moving operand): no PE
  transposes, no PSUM->SBUF copies.
- Act packing per chain-step: sigmoid(bank A) [512]; tanh(gB*0.5) split
  as tanh-g [256] (feeds ig early) + tanh-o [256], with g-gate weights
  PRE-DOUBLED on host so tanh(o/2) -> sigmoid(o) via a 0.5*t+0.5 affine
  on Pool (off the critical path) while tanh(g) is exact; tanh(c) [256].
- DVE: fc, ig, c_new, h-mul halves (bf16 out). Pool: sigma(o) affine.
- Phase 2: final linear groups (bf16 moving, 512-wide) emitted INSIDE the
  loop as soon as both chains have written the group's X positions; bias
  via DVE tensor_scalar; single contiguous DMA per group on alternating
  queues. PE p-state pre-warm burns the 3us ramp during weight DMAs.
"""

import sys
import numpy as np
from contextlib import ExitStack

sys.path.insert(0, "/opt/trn_rl_repo")

import concourse.bass as bass
import concourse.tile as tile
from concourse import bacc, mybir
from concourse.bass_utils import run_bass_kernel_spmd

# ---- problem constants (hardcoded per contract) ----
VOCAB, EMB, HID, MEL = 256, 128, 256, 80
B, T = 64, 512
N_CORES = 8
NCHUNK = 16          # chunks per direction
W = 11               # warmup steps per chain
CHUNK = 43           # positions per chunk; L_PAD = 688 >= L
L_PAD = NCHUNK * CHUNK
K_STEPS = W + CHUNK
CHUNK2 = 2 * CHUNK   # positions per core
G4 = 4 * HID         # 1024
F32 = mybir.dt.float32
BF16 = mybir.dt.bfloat16
SIG = mybir.ActivationFunctionType.Sigmoid
TANH = mybir.ActivationFunctionType.Tanh
IDENT = mybir.ActivationFunctionType.Identity
MULT = mybir.AluOpType.mult
ADD = mybir.AluOpType.add

_COMPILED = None
WARMERS = False


def _host_expand(x, embed, dp_w, dp_b):
    xe = embed[x]                                   # (B,T,E)
    d = np.maximum(xe @ dp_w[0] + dp_b[0], 0)
    dur = np.floor(d).astype(np.int64) + 1
    cum = np.cumsum(dur, axis=1)
    L = int(cum[:, -1].max())
    pos = np.arange(L)
    idx = np.empty((B, L), np.int64)
    for b in range(B):
        idx[b] = np.searchsorted(cum[b], pos, side="right")
    mask = (pos[None, :] < cum[:, -1:]).astype(np.float32)
    exp = np.take_along_axis(xe, np.clip(idx, 0, T - 1)[..., None], axis=1)
    return np.ascontiguousarray(exp * mask[..., None], dtype=np.float32), L


# m-chunk order [f0 f1 i0 i1 o0 o1 g0 g1]; rows in PyTorch [i,f,g,o] layout.
# g rows are doubled so tanh(pre*0.5) gives tanh(g) while o gets tanh(o/2).
def _mchunk_rows():
    rows, scale = [], []
    for base, sc in ((HID, 1.0), (0, 1.0), (3 * HID, 1.0), (2 * HID, 2.0)):
        for half in (0, 1):
            rows.append(np.arange(base + half * 128, base + half * 128 + 128))
            scale.append(np.full(128, sc, np.float32))
    return np.concatenate(rows), np.concatenate(scale)


class _Chain:
    def __init__(self, name, whh, wih, xe_cols, X, poolA, poolB):
        self.name = name
        self.whh = whh          # sbuf [128, 16*128] bf16, tile (m,k) at (2m+k)*128
        self.wih = wih          # sbuf [128, 8*128] bf16, tile m at m*128
        self.xe_cols = xe_cols  # slice in the xein tile
        self.X = X              # sbuf [128, (CHUNK+2)*256] bf16
        self.poolA = poolA      # PSUM pool for bank A (f,i gates)
        self.poolB = poolB      # PSUM pool for bank B (o,g gates)
        self.gA = None
        self.gB = None
        self.src = None         # X col block [128, 256] (h of prev step)
        self.c_prev = None


def _build_kernel():
    nc = bacc.Bacc("TRN2", target_bir_lowering=False, debug=False,
                   num_devices=N_CORES)

    xein = nc.dram_tensor("xein", [K_STEPS, EMB, 256], BF16,
                          kind="ExternalInput").ap()
    whh_f_d = nc.dram_tensor("whhT_f", [128, 16 * 128], BF16, kind="ExternalInput").ap()
    whh_b_d = nc.dram_tensor("whhT_b", [128, 16 * 128], BF16, kind="ExternalInput").ap()
    wih_f_d = nc.dram_tensor("wihT_f", [128, 8 * 128], BF16, kind="ExternalInput").ap()
    wih_b_d = nc.dram_tensor("wihT_b", [128, 8 * 128], BF16, kind="ExternalInput").ap()
    lin_w_d = nc.dram_tensor("linT", [128, 4 * MEL], BF16, kind="ExternalInput").ap()
    lin_b_d = nc.dram_tensor("lin_b", [MEL, 1], F32, kind="ExternalInput").ap()
    out_d = nc.dram_tensor("out_mel", [MEL, CHUNK, 2, B], F32,
                           kind="ExternalOutput").ap()

    with tile.TileContext(nc) as tc, ExitStack() as ctx:
        wpool = ctx.enter_context(tc.tile_pool(name="weights", bufs=1))
        xpool = ctx.enter_context(tc.tile_pool(name="xstream", bufs=5))
        state = ctx.enter_context(tc.tile_pool(name="state", bufs=4))
        actp = ctx.enter_context(tc.tile_pool(name="acts", bufs=6))
        xbig = ctx.enter_context(tc.tile_pool(name="xbig", bufs=1))
        scr = ctx.enter_context(tc.tile_pool(name="scratch", bufs=6))
        gAf = ctx.enter_context(tc.tile_pool(name="gAf", bufs=2, space="PSUM"))
        gBf = ctx.enter_context(tc.tile_pool(name="gBf", bufs=2, space="PSUM"))
        gAb = ctx.enter_context(tc.tile_pool(name="gAb", bufs=2, space="PSUM"))
        gBb = ctx.enter_context(tc.tile_pool(name="gBb", bufs=2, space="PSUM"))
        ostage = ctx.enter_context(tc.tile_pool(name="ostage", bufs=2))

        # ---- memsets first (Pool queue) so the PE pre-warm starts at t~0
        hinit = wpool.tile([128, 256], BF16, tag="hinit")
        nc.gpsimd.memset(hinit[:], 0.0)
        zstat = wpool.tile([128, 64], F32, tag="zstat")
        nc.gpsimd.memset(zstat[:], 0.0)
        zstat_bf = wpool.tile([128, 64], BF16, tag="zstatbf")
        nc.gpsimd.memset(zstat_bf[:], 0.0)

        # PE p-state pre-warm: burn the ramp on dummy matmuls while the
        # weight DMAs are in flight, so step 0 runs at full clock.
        warm = gBb.tile([128, 512], F32, tag="g", name="pewarm")
        NWARM = 10
        for i in range(NWARM):
            nc.tensor.matmul(warm[0:64, 0:256], zstat_bf[:], hinit[:],
                             start=(i == 0), stop=(i == NWARM - 1))

        # ---- first xe stream DMAs go ahead of the big weight DMAs ----
        xe_tiles = {}

        def emit_dma(s):
            if s not in xe_tiles and s < K_STEPS:
                xe = xpool.tile([EMB, 256], BF16, tag="xe", name=f"xe{s}")
                nc.sync.dma_start(xe[:], xein[s])
                xe_tiles[s] = xe

        emit_dma(0)
        emit_dma(1)

        # ---- weights -> SBUF (one DMA per tensor; host lays out tiles).
        # Order: what step 0 needs first (wih for xe mms, then whh).
        wih_f = wpool.tile([128, 8 * 128], BF16, tag="wihf")
        nc.scalar.dma_start(wih_f[:], wih_f_d[:])
        wih_b = wpool.tile([128, 8 * 128], BF16, tag="wihb")
        nc.gpsimd.dma_start(wih_b[:], wih_b_d[:])
        whh_f = wpool.tile([128, 16 * 128], BF16, tag="whhf")
        nc.sync.dma_start(whh_f[:], whh_f_d[:])
        whh_b = wpool.tile([128, 16 * 128], BF16, tag="whhb")
        nc.scalar.dma_start(whh_b[:], whh_b_d[:])
        lin_w = wpool.tile([128, 4 * MEL], BF16, tag="linw")
        nc.scalar.dma_start(lin_w[:], lin_w_d[:])
        lin_b = wpool.tile([MEL, 1], F32, tag="linb")
        nc.gpsimd.dma_start(lin_b[:], lin_b_d[:])

        XW = (CHUNK + 2) * 128
        X_f = [xbig.tile([128, XW], BF16, tag=f"Xf{k}", name=f"Xf{k}")
               for k in (0, 1)]
        X_b = [xbig.tile([128, XW], BF16, tag=f"Xb{k}", name=f"Xb{k}")
               for k in (0, 1)]

        chains = [
            _Chain("f", whh_f, wih_f, slice(0, 128), X_f, gAf, gBf),
            _Chain("b", whh_b, wih_b, slice(128, 256), X_b, gAb, gBb),
        ]
        for ch in chains:
            ch.src = (hinit[:, 0:128], hinit[:, 128:256])
            c0 = state.tile([128, 256], F32, tag="c" + ch.name,
                            name=f"c0{ch.name}")
            nc.gpsimd.memset(c0[:], 0.0)
            ch.c_prev = c0

        def emit_xe_mms(ch, s, close=False):
            # close=True: no recurrent mms will follow (h_prev == 0 exactly
            # at s=0, so W_hh @ h contributes nothing) -> stop the groups.
            emit_dma(s)
            xe = xe_tiles[s]
            gA = ch.poolA.tile([128, 512], F32, tag="g", name=f"gA{ch.name}{s}")
            gB = ch.poolB.tile([128, 512], F32, tag="g", name=f"gB{ch.name}{s}")
            for m in range(8):
                g = gA if m < 4 else gB
                col = (m % 4) * 128
                nc.tensor.matmul(g[:, col:col + 128],
                                 ch.wih[:, m * 128:(m + 1) * 128],
                                 xe[:, ch.xe_cols],
                                 start=(m in (0, 4)),
                                 stop=(close and m in (3, 7)))
            return gA, gB

        def emit_rec_mms(ch, bank):
            # bank 0: m-chunks 0..3 (f,i); bank 1: m-chunks 4..7 (o,g).
            # k-major: all k0 mms first (they only need the h0 half).
            g = ch.gA if bank == 0 else ch.gB
            for k in (0, 1):
                for m in range(bank * 4, bank * 4 + 4):
                    last = (m == bank * 4 + 3) and (k == 1)
                    col = (m % 4) * 128
                    nc.tensor.matmul(
                        g[:, col:col + 128],
                        ch.whh[:, (2 * m + k) * 128:(2 * m + k + 1) * 128],
                        ch.src[k],
                        start=False, stop=last)

        for ch in chains:
            ch.gA, ch.gB = emit_xe_mms(ch, 0, close=True)
        emit_dma(1)

        # ---- phase 2 (final linear) groups, interleaved into the loop as
        # soon as both chains have written X for the group's positions ----
        movs = [X[k][:, 0:CHUNK * 128].rearrange("p (t l) -> p t l", l=128)
                for X in (X_f, X_b) for k in (0, 1)]
        gstate = {"gi": 0}

        def emit_group(p0, glen):
            n = glen * 128
            ps = gAf.tile([MEL, 512], F32, tag="g", name=f"op{p0}")
            for k in range(4):
                nc.tensor.matmul(ps[:, 0:n], lin_w[:, k * MEL:(k + 1) * MEL],
                                 movs[k][:, p0:p0 + glen],
                                 start=(k == 0), stop=(k == 3))
            o_sb = ostage.tile([MEL, 512], F32, tag="os", name=f"os{p0}")
            nc.vector.tensor_scalar(o_sb[:, 0:n], ps[:, 0:n], lin_b[:], None,
                                    ADD)
            q = (nc.sync, nc.gpsimd)[gstate["gi"] % 2]
            q.dma_start(out_d[:, p0:p0 + glen], o_sb[:, 0:n])
            gstate["gi"] += 1

        groups_at = {}
        p0 = 0
        while p0 < CHUNK:
            glen = min(4, CHUNK - p0)
            ready = W + max(p0 + glen - 1, CHUNK - 1 - p0)
            groups_at.setdefault(min(ready, K_STEPS - 1), []).append((p0, glen))
            p0 += glen

        for s in range(K_STEPS):
            real = s >= W
            t_rel = s - W

            # --- recurrent matmuls; leading chain alternates per step.
            # s=0 has h_prev == 0: the xe mms already closed the groups. ---
            ch0, ch1 = (chains if (s // 3) % 2 == 0 else (chains[1], chains[0]))
            if s > 0:
                emit_rec_mms(ch0, 0)
                emit_rec_mms(ch0, 1)
            gates_next = {}
            if s + 1 < K_STEPS:
                gates_next[ch0.name] = emit_xe_mms(ch0, s + 1)
            if s > 0:
                emit_rec_mms(ch1, 0)
                emit_rec_mms(ch1, 1)
            if s + 1 < K_STEPS:
                gates_next[ch1.name] = emit_xe_mms(ch1, s + 1)
            emit_dma(s + 2)
            emit_dma(s + 3)

            # --- pointwise ---
            def emit_sf(ch):
                nm = f"{ch.name}{s}"
                sf = actp.tile([128, 512], F32, tag="sf" + ch.name, name="sf" + nm)
                nc.scalar.activation(sf[:], ch.gA[:], SIG)
                return sf

            def emit_tog_g(ch):
                nm = f"{ch.name}{s}"
                tg = actp.tile([128, 256], F32, tag="tg" + ch.name,
                               name="tg" + nm)
                nc.scalar.activation(tg[:], ch.gB[:, 256:512], TANH, scale=0.5)
                return tg

            def emit_tog_o(ch):
                nm = f"{ch.name}{s}"
                to = actp.tile([128, 256], F32, tag="to" + ch.name,
                               name="to" + nm)
                nc.scalar.activation(to[:], ch.gB[:, 0:256], TANH, scale=0.5)
                return to

            def emit_mid(ch, sf, tg, to):
                nm = f"{ch.name}{s}"
                # PE p-state warmer: zero-contribution matmul anchored on sf
                # fires mid-gap and keeps the activity window alive.
                if WARMERS and s + 1 < K_STEPS:
                    nc.tensor.matmul(gates_next[ch.name][0][0:64, 0:128],
                                     zstat[:], sf[:, 0:128],
                                     start=False, stop=False,
                                     skip_group_check=True)
                fc = scr.tile([128, 256], F32, tag="fc" + ch.name, name="fc" + nm)
                nc.vector.tensor_mul(fc[:], sf[:, 0:256], ch.c_prev[:])
                so = scr.tile([128, 256], F32, tag="so" + ch.name, name="so" + nm)
                nc.gpsimd.tensor_scalar(so[:], to[:], 0.5, 0.5, MULT, ADD)
                ig = scr.tile([128, 256], F32, tag="ig" + ch.name, name="ig" + nm)
                nc.vector.tensor_mul(ig[:], sf[:, 256:512], tg[:])
                c_new = state.tile([128, 256], F32, tag="c" + ch.name,
                                   name="c" + nm)
                nc.vector.tensor_add(c_new[:], fc[:], ig[:])
                return so, c_new

            def emit_tc(ch, c_new):
                nm = f"{ch.name}{s}"
                tc_ = actp.tile([128, 256], F32, tag="tc" + ch.name,
                                name="tc" + nm)
                nc.scalar.activation(tc_[:], c_new[:], TANH)
                return tc_

            def emit_h(ch, so, tc_, c_new):
                if real:
                    lp = t_rel if ch.name == "f" else CHUNK - 1 - t_rel
                else:
                    lp = CHUNK + (s & 1)
                dst = tuple(ch.X[k][:, lp * 128:(lp + 1) * 128] for k in (0, 1))
                nc.vector.tensor_mul(dst[0], so[:, 0:128], tc_[:, 0:128])
                nc.vector.tensor_mul(dst[1], so[:, 128:256], tc_[:, 128:256])
                # second warmer anchored on tc
                if WARMERS and s + 1 < K_STEPS:
                    nc.tensor.matmul(gates_next[ch.name][1][0:64, 0:128],
                                     zstat[:], tc_[:, 0:128],
                                     start=False, stop=False,
                                     skip_group_check=True)
                ch.src = dst
                ch.c_prev = c_new
                if s + 1 < K_STEPS:
                    ch.gA, ch.gB = gates_next[ch.name]

            chf, chb = (chains if (s // 3) % 2 == 0 else (chains[1], chains[0]))
            sf_f = emit_sf(chf)
            tg_f = emit_tog_g(chf)
            to_f = emit_tog_o(chf)
            sf_b = emit_sf(chb)
            so_f, c_f = emit_mid(chf, sf_f, tg_f, to_f)
            tc_f = emit_tc(chf, c_f)
            tg_b = emit_tog_g(chb)
            to_b = emit_tog_o(chb)
            emit_h(chf, so_f, tc_f, c_f)
            so_b, c_b = emit_mid(chb, sf_b, tg_b, to_b)
            tc_b = emit_tc(chb, c_b)
            emit_h(chb, so_b, tc_b, c_b)

            for (p0g, gl) in groups_at.get(s, []):
                emit_group(p0g, gl)



    nc.compile()
    return nc


def _np_lstm_fallback(exp, inputs):
    def sigmoid(z):
        return 1.0 / (1.0 + np.exp(-z))

    def lstm(xs, wih, whh, bih, bhh):
        Bb, L, E = xs.shape
        pre = np.einsum("ble,ge->blg", xs, wih) + bih + bhh
        h = np.zeros((Bb, HID), np.float32)
        c = np.zeros((Bb, HID), np.float32)
        hs = np.zeros((Bb, L, HID), np.float32)
        for t in range(L):
            gg = pre[:, t] + h @ whh.T
            i, f, g_, o = np.split(gg, 4, axis=-1)
            c = sigmoid(f) * c + sigmoid(i) * np.tanh(g_)
            h = sigmoid(o) * np.tanh(c)
            hs[:, t] = h
        return hs

    out_f = lstm(exp, inputs["wih_f"], inputs["whh_f"], inputs["bih_f"],
                 inputs["bhh_f"])
    out_b = lstm(exp[:, ::-1], inputs["wih_b"], inputs["whh_b"],
                 inputs["bih_b"], inputs["bhh_b"])[:, ::-1]
    out = np.concatenate([out_f, out_b], axis=-1)
    return out @ inputs["lin_w"].T + inputs["lin_b"]


def make_in_maps(expP, expR, inputs):
    import ml_dtypes
    bf16 = ml_dtypes.bfloat16
    rows, scale = _mchunk_rows()

    def stat_tiles(w):
        # sbuf layout [128, ntiles*128]: tile (m,k) at cols (nk*m+k)*128
        wp = (w.astype(np.float32)[rows] * scale[:, None])
        nk = w.shape[1] // 128
        out = np.zeros((128, 8 * nk * 128), np.float32)
        for m in range(8):
            for k in range(nk):
                out[:, (m * nk + k) * 128:(m * nk + k + 1) * 128] = \
                    wp[m * 128:(m + 1) * 128, k * 128:(k + 1) * 128].T
        return np.ascontiguousarray(out).astype(bf16)

    whhT_f = stat_tiles(inputs["whh_f"])
    whhT_b = stat_tiles(inputs["whh_b"])
    wihT_f = stat_tiles(inputs["wih_f"])
    wihT_b = stat_tiles(inputs["wih_b"])
    lw = inputs["lin_w"].astype(np.float32)
    linT = np.concatenate([np.ascontiguousarray(lw[:, k * 128:(k + 1) * 128].T)
                           for k in range(4)], axis=1).astype(bf16)
    lin_b2 = np.ascontiguousarray(inputs["lin_b"].astype(np.float32)[:, None])

    in_maps = []
    for j in range(N_CORES):
        xein = np.zeros((K_STEPS, EMB, 256), np.float32)
        starts = [2 * j * CHUNK - W,
                  (2 * j + 1) * CHUNK - W,
                  (15 - 2 * j) * CHUNK - W,
                  (14 - 2 * j) * CHUNK - W]
        srcs = [expP, expP, expR, expR]
        for s in range(K_STEPS):
            for ci, (st, src) in enumerate(zip(starts, srcs)):
                p = st + s
                if 0 <= p < L_PAD:
                    xein[s, :, ci * 64:(ci + 1) * 64] = src[:, p].T
        in_maps.append({
            "xein": xein.astype(bf16),
            "whhT_f": whhT_f, "whhT_b": whhT_b,
            "wihT_f": wihT_f, "wihT_b": wihT_b,
            "linT": linT, "lin_b": lin_b2,
        })
    return in_maps


def kernel(**inputs):
    global _COMPILED
    inputs = {k: np.asarray(v) for k, v in inputs.items()}
    x = inputs["x"].astype(np.int64)
    exp, L = _host_expand(x, inputs["embed"].astype(np.float32),
                          inputs["dp_w"].astype(np.float32),
                          inputs["dp_b"].astype(np.float32))

    bias_mag = max(float(np.abs(inputs[k]).max())
                   for k in ("bih_f", "bhh_f", "bih_b", "bhh_b"))
    if L > L_PAD or bias_mag != 0.0:
        f32in = {k: (v.astype(np.float32) if v.dtype.kind == "f" else v)
                 for k, v in inputs.items()}
        return _np_lstm_fallback(exp, f32in).astype(np.float32)

    expP = np.zeros((B, L_PAD, EMB), np.float32)
    expP[:, :L] = exp
    expR = expP[:, ::-1]

    in_maps = make_in_maps(expP, expR, inputs)

    if _COMPILED is None:
        _COMPILED = _build_kernel()
    nc = _COMPILED

    res = run_bass_kernel_spmd(nc, in_maps, core_ids=list(range(N_CORES)))

    out = np.empty((B, L_PAD, MEL), np.float32)
    for j in range(N_CORES):
        om = res.results[j]["out_mel"]          # [MEL, CHUNK, 2, B]
        blk = om.transpose(3, 2, 1, 0).reshape(B, CHUNK2, MEL)
        out[:, j * CHUNK2:(j + 1) * CHUNK2] = blk
    return np.ascontiguousarray(out[:, :L])


if __name__ == "__main__":
    inputs = dict(np.load("/root/problem/inputs.npz"))
    out = kernel(**inputs)
    ref = np.load("/root/problem/expected.npy")
    diff = np.abs(out - ref)
    print("out", out.shape, "absmax diff", diff.max(),
          "rel", diff.max() / np.abs(ref).max())


# revision 53
# speedup vs baseline: 1.0137x; 1.0137x over previous
"""MiniFastSpeech Trainium2 kernel (v2: weight-stationary bf16 LSTM).

Strategy:
- Host (numpy): embed lookup, duration predictor, cumsum, searchsorted
  length-regulator expansion -> exp [B, L, E]; pad to L_PAD = 16*CHUNK.
- Device (8 cores, SPMD): bidirectional LSTM via sequence-chunked
  parallelism with zero-state warmup (state sensitivity decays ~0.75/step;
  W=11 measured 7.8e-3 output rel err on HW, budget 2e-2).
- Core j runs two lockstep pair-chains (fwd chunks (2j,2j+1), bwd chunks
  (15-2j,14-2j) over the reversed sequence), 128 lanes = 64 batch x 2
  chunks each.
- Weight-stationary layout: gates live in two PSUM tiles per chain
  (bank A = [f0 f1 i0 i1], bank B = [o0 o1 g0 g1]; each [128 part =
  gate-dim-chunk, 4 m-chunks x 128 lanes]) so sigmoid(A) only waits on
  A's 8 recurrent matmuls. Recurrent matmuls: stationary = whh tile
  [128 hid-k, 128 gate-m] bf16, moving = h tile [128 hid-k, 128 lanes]
  bf16 (1 cyc/row at any size), k-major order so k0 mms start right
  after the h0 half-write. h is written ONCE per step as bf16 directly
  into the per-k-chunk X accumulators (which double as the h stream for
  the next step's moving operand and phase-2's moving operand): no PE
  transposes, no PSUM->SBUF copies.
- Act packing per chain-step: sigmoid(bank A) [512]; tanh(gB*0.5) split
  as tanh-g [256] (feeds ig early) + tanh-o [256], with g-gate weights
  PRE-DOUBLED on host so tanh(o/2) -> sigmoid(o) via a 0.5*t+0.5 affine
  on Pool (off the critical path) while tanh(g) is exact; tanh(c) [256].
- DVE: fc, ig, c_new, h-mul halves (bf16 out). Pool: sigma(o) affine.
- Phase 2: final linear groups (bf16 moving, 512-wide) emitted INSIDE the
  loop as soon as both chains have written the group's X positions; bias
  via DVE tensor_scalar; single contiguous DMA per group on alternating
  queues. PE p-state pre-warm burns the 3us ramp during weight DMAs.
"""

import sys
import numpy as np
from contextlib import ExitStack

sys.path.insert(0, "/opt/trn_rl_repo")

import concourse.bass as bass
import concourse.tile as tile
from concourse import bacc, mybir
from concourse.bass_utils import run_bass_kernel_spmd

# ---- problem constants (hardcoded per contract) ----
VOCAB, EMB, HID, MEL = 256, 128, 256, 80
B, T = 64, 512
N_CORES = 8
NCHUNK = 16          # chunks per direction
W = 11               # warmup steps per chain
CHUNK = 43           # positions per chunk; L_PAD = 688 >= L
L_PAD = NCHUNK * CHUNK
K_STEPS = W + CHUNK
CHUNK2 = 2 * CHUNK   # positions per core
G4 = 4 * HID         # 1024
F32 = mybir.dt.float32
BF16 = mybir.dt.bfloat16
SIG = mybir.ActivationFunctionType.Sigmoid
TANH = mybir.ActivationFunctionType.Tanh
IDENT = mybir.ActivationFunctionType.Identity
MULT = mybir.AluOpType.mult
ADD = mybir.AluOpType.add

_COMPILED = None
WARMERS = False


def _host_expand(x, embed, dp_w, dp_b):
    xe = embed[x]                                   # (B,T,E)
    d = np.maximum(xe @ dp_w[0] + dp_b[0], 0)
    dur = np.floor(d).astype(np.int64) + 1
    cum = np.cumsum(dur, axis=1)
    L = int(cum[:, -1].max())
    pos = np.arange(L)
    idx = np.empty((B, L), np.int64)
    for b in range(B):
        idx[b] = np.searchsorted(cum[b], pos, side="right")
    mask = (pos[None, :] < cum[:, -1:]).astype(np.float32)
    exp = np.take_along_axis(xe, np.clip(idx, 0, T - 1)[..., None], axis=1)
    return np.ascontiguousarray(exp * mask[..., None], dtype=np.float32), L


# m-chunk order [f0 f1 i0 i1 o0 o1 g0 g1]; rows in PyTorch [i,f,g,o] layout.
# g rows are doubled so tanh(pre*0.5) gives tanh(g) while o gets tanh(o/2).
def _mchunk_rows():
    rows, scale = [], []
    for base, sc in ((HID, 1.0), (0, 1.0), (3 * HID, 1.0), (2 * HID, 2.0)):
        for half in (0, 1):
            rows.append(np.arange(base + half * 128, base + half * 128 + 128))
            scale.append(np.full(128, sc, np.float32))
    return np.concatenate(rows), np.concatenate(scale)


class _Chain:
    def __init__(self, name, whh, wih, xe_cols, X, poolA, poolB):
        self.name = name
        self.whh = whh          # sbuf [128, 16*128] bf16, tile (m,k) at (2m+k)*128
        self.wih = wih          # sbuf [128, 8*128] bf16, tile m at m*128
        self.xe_cols = xe_cols  # slice in the xein tile
        self.X = X              # sbuf [128, (CHUNK+2)*256] bf16
        self.poolA = poolA      # PSUM pool for bank A (f,i gates)
        self.poolB = poolB      # PSUM pool for bank B (o,g gates)
        self.gA = None
        self.gB = None
        self.src = None         # X col block [128, 256] (h of prev step)
        self.c_prev = None


def _build_kernel():
    nc = bacc.Bacc("TRN2", target_bir_lowering=False, debug=False,
                   num_devices=N_CORES)

    xein = nc.dram_tensor("xein", [K_STEPS, EMB, 256], BF16,
                          kind="ExternalInput").ap()
    whh_f_d = nc.dram_tensor("whhT_f", [128, 16 * 128], BF16, kind="ExternalInput").ap()
    whh_b_d = nc.dram_tensor("whhT_b", [128, 16 * 128], BF16, kind="ExternalInput").ap()
    wih_f_d = nc.dram_tensor("wihT_f", [128, 8 * 128], BF16, kind="ExternalInput").ap()
    wih_b_d = nc.dram_tensor("wihT_b", [128, 8 * 128], BF16, kind="ExternalInput").ap()
    lin_w_d = nc.dram_tensor("linT", [128, 4 * MEL], BF16, kind="ExternalInput").ap()
    lin_b_d = nc.dram_tensor("lin_b", [MEL, 1], F32, kind="ExternalInput").ap()
    out_d = nc.dram_tensor("out_mel", [MEL, CHUNK, 2, B], F32,
                           kind="ExternalOutput").ap()

    with tile.TileContext(nc) as tc, ExitStack() as ctx:
        wpool = ctx.enter_context(tc.tile_pool(name="weights", bufs=1))
        xpool = ctx.enter_context(tc.tile_pool(name="xstream", bufs=5))
        state = ctx.enter_context(tc.tile_pool(name="state", bufs=4))
        actp = ctx.enter_context(tc.tile_pool(name="acts", bufs=6))
        xbig = ctx.enter_context(tc.tile_pool(name="xbig", bufs=1))
        scr = ctx.enter_context(tc.tile_pool(name="scratch", bufs=6))
        gAf = ctx.enter_context(tc.tile_pool(name="gAf", bufs=2, space="PSUM"))
        gBf = ctx.enter_context(tc.tile_pool(name="gBf", bufs=2, space="PSUM"))
        gAb = ctx.enter_context(tc.tile_pool(name="gAb", bufs=2, space="PSUM"))
        gBb = ctx.enter_context(tc.tile_pool(name="gBb", bufs=2, space="PSUM"))
        ostage = ctx.enter_context(tc.tile_pool(name="ostage", bufs=2))

        # ---- memsets first (Pool queue) so the PE pre-warm starts at t~0
        hinit = wpool.tile([128, 256], BF16, tag="hinit")
        nc.gpsimd.memset(hinit[:], 0.0)
        zstat = wpool.tile([128, 64], F32, tag="zstat")
        nc.gpsimd.memset(zstat[:], 0.0)
        zstat_bf = wpool.tile([128, 64], BF16, tag="zstatbf")
        nc.gpsimd.memset(zstat_bf[:], 0.0)

        # PE p-state pre-warm: burn the ramp on dummy matmuls while the
        # weight DMAs are in flight, so step 0 runs at full clock.
        warm = gBb.tile([128, 512], F32, tag="g", name="pewarm")
        NWARM = 10
        for i in range(NWARM):
            nc.tensor.matmul(warm[0:64, 0:256], zstat_bf[:], hinit[:],
                             start=(i == 0), stop=(i == NWARM - 1))

        # ---- first xe stream DMAs go ahead of the big weight DMAs ----
        xe_tiles = {}

        def emit_dma(s):
            if s not in xe_tiles and s < K_STEPS:
                xe = xpool.tile([EMB, 256], BF16, tag="xe", name=f"xe{s}")
                nc.sync.dma_start(xe[:], xein[s])
                xe_tiles[s] = xe

        emit_dma(0)
        emit_dma(1)

        # ---- weights -> SBUF (one DMA per tensor; host lays out tiles).
        # Order: what step 0 needs first (wih for xe mms, then whh).
        wih_f = wpool.tile([128, 8 * 128], BF16, tag="wihf")
        nc.scalar.dma_start(wih_f[:], wih_f_d[:])
        wih_b = wpool.tile([128, 8 * 128], BF16, tag="wihb")
        nc.gpsimd.dma_start(wih_b[:], wih_b_d[:])
        whh_f = wpool.tile([128, 16 * 128], BF16, tag="whhf")
        nc.sync.dma_start(whh_f[:], whh_f_d[:])
        whh_b = wpool.tile([128, 16 * 128], BF16, tag="whhb")
        nc.scalar.dma_start(whh_b[:], whh_b_d[:])
        lin_w = wpool.tile([128, 4 * MEL], BF16, tag="linw")
        nc.scalar.dma_start(lin_w[:], lin_w_d[:])
        lin_b = wpool.tile([MEL, 1], F32, tag="linb")
        nc.gpsimd.dma_start(lin_b[:], lin_b_d[:])

        XW = (CHUNK + 2) * 128
        X_f = [xbig.tile([128, XW], BF16, tag=f"Xf{k}", name=f"Xf{k}")
               for k in (0, 1)]
        X_b = [xbig.tile([128, XW], BF16, tag=f"Xb{k}", name=f"Xb{k}")
               for k in (0, 1)]

        chains = [
            _Chain("f", whh_f, wih_f, slice(0, 128), X_f, gAf, gBf),
            _Chain("b", whh_b, wih_b, slice(128, 256), X_b, gAb, gBb),
        ]
        for ch in chains:
            ch.src = (hinit[:, 0:128], hinit[:, 128:256])
            c0 = state.tile([128, 256], F32, tag="c" + ch.name,
                            name=f"c0{ch.name}")
            nc.gpsimd.memset(c0[:], 0.0)
            ch.c_prev = c0

        def emit_xe_mms(ch, s, close=False):
            # close=True: no recurrent mms will follow (h_prev == 0 exactly
            # at s=0, so W_hh @ h contributes nothing) -> stop the groups.
            emit_dma(s)
            xe = xe_tiles[s]
            gA = ch.poolA.tile([128, 512], F32, tag="g", name=f"gA{ch.name}{s}")
            gB = ch.poolB.tile([128, 512], F32, tag="g", name=f"gB{ch.name}{s}")
            for m in range(8):
                g = gA if m < 4 else gB
                col = (m % 4) * 128
                nc.tensor.matmul(g[:, col:col + 128],
                                 ch.wih[:, m * 128:(m + 1) * 128],
                                 xe[:, ch.xe_cols],
                                 start=(m in (0, 4)),
                                 stop=(close and m in (3, 7)))
            return gA, gB

        def emit_rec_mms(ch, bank):
            # bank 0: m-chunks 0..3 (f,i); bank 1: m-chunks 4..7 (o,g).
            # k-major: all k0 mms first (they only need the h0 half).
            g = ch.gA if bank == 0 else ch.gB
            for k in (0, 1):
                for m in range(bank * 4, bank * 4 + 4):
                    last = (m == bank * 4 + 3) and (k == 1)
                    col = (m % 4) * 128
                    nc.tensor.matmul(
                        g[:, col:col + 128],
                        ch.whh[:, (2 * m + k) * 128:(2 * m + k + 1) * 128],
                        ch.src[k],
                        start=False, stop=last)

        for ch in chains:
            ch.gA, ch.gB = emit_xe_mms(ch, 0, close=True)
        emit_dma(1)

        # ---- phase 2 (final linear) groups, interleaved into the loop as
        # soon as both chains have written X for the group's positions ----
        movs = [X[k][:, 0:CHUNK * 128].rearrange("p (t l) -> p t l", l=128)
                for X in (X_f, X_b) for k in (0, 1)]
        gstate = {"gi": 0}

        def emit_group(p0, glen):
            n = glen * 128
            ps = gAf.tile([MEL, 512], F32, tag="g", name=f"op{p0}")
            for k in range(4):
                nc.tensor.matmul(ps[:, 0:n], lin_w[:, k * MEL:(k + 1) * MEL],
                                 movs[k][:, p0:p0 + glen],
                                 start=(k == 0), stop=(k == 3))
            o_sb = ostage.tile([MEL, 512], F32, tag="os", name=f"os{p0}")
            nc.vector.tensor_scalar(o_sb[:, 0:n], ps[:, 0:n], lin_b[:], None,
                                    ADD)
            q = (nc.sync, nc.gpsimd)[gstate["gi"] % 2]
            q.dma_start(out_d[:, p0:p0 + glen], o_sb[:, 0:n])
            gstate["gi"] += 1

        groups_at = {}
        p0 = 0
        while p0 < CHUNK:
            glen = min(4, CHUNK - p0)
            ready = W + max(p0 + glen - 1, CHUNK - 1 - p0)
            groups_at.setdefault(min(ready, K_STEPS - 1), []).append((p0, glen))
            p0 += glen

        for s in range(K_STEPS):
            real = s >= W
            t_rel = s - W

            # --- recurrent matmuls; leading chain alternates per step.
            # s=0 has h_prev == 0: the xe mms already closed the groups. ---
            ch0, ch1 = (chains if (s // 2) % 2 == 0 else (chains[1], chains[0]))
            if s > 0:
                emit_rec_mms(ch0, 0)
                emit_rec_mms(ch0, 1)
            gates_next = {}
            if s + 1 < K_STEPS:
                gates_next[ch0.name] = emit_xe_mms(ch0, s + 1)
            if s > 0:
                emit_rec_mms(ch1, 0)
                emit_rec_mms(ch1, 1)
            if s + 1 < K_STEPS:
                gates_next[ch1.name] = emit_xe_mms(ch1, s + 1)
            emit_dma(s + 2)
            emit_dma(s + 3)

            # --- pointwise ---
            def emit_sf(ch):
                nm = f"{ch.name}{s}"
                sf = actp.tile([128, 512], F32, tag="sf" + ch.name, name="sf" + nm)
                nc.scalar.activation(sf[:], ch.gA[:], SIG)
                return sf

            def emit_tog_g(ch):
                nm = f"{ch.name}{s}"
                tg = actp.tile([128, 256], F32, tag="tg" + ch.name,
                               name="tg" + nm)
                nc.scalar.activation(tg[:], ch.gB[:, 256:512], TANH, scale=0.5)
                return tg

            def emit_tog_o(ch):
                nm = f"{ch.name}{s}"
                to = actp.tile([128, 256], F32, tag="to" + ch.name,
                               name="to" + nm)
                nc.scalar.activation(to[:], ch.gB[:, 0:256], TANH, scale=0.5)
                return to

            def emit_mid(ch, sf, tg, to):
                nm = f"{ch.name}{s}"
                # PE p-state warmer: zero-contribution matmul anchored on sf
                # fires mid-gap and keeps the activity window alive.
                if WARMERS and s + 1 < K_STEPS:
                    nc.tensor.matmul(gates_next[ch.name][0][0:64, 0:128],
                                     zstat[:], sf[:, 0:128],
                                     start=False, stop=False,
                                     skip_group_check=True)
                fc = scr.tile([128, 256], F32, tag="fc" + ch.name, name="fc" + nm)
                nc.vector.tensor_mul(fc[:], sf[:, 0:256], ch.c_prev[:])
                so = scr.tile([128, 256], F32, tag="so" + ch.name, name="so" + nm)
                nc.gpsimd.tensor_scalar(so[:], to[:], 0.5, 0.5, MULT, ADD)
                ig = scr.tile([128, 256], F32, tag="ig" + ch.name, name="ig" + nm)
                nc.vector.tensor_mul(ig[:], sf[:, 256:512], tg[:])
                c_new = state.tile([128, 256], F32, tag="c" + ch.name,
                                   name="c" + nm)
                nc.vector.tensor_add(c_new[:], fc[:], ig[:])
                return so, c_new

            def emit_tc(ch, c_new):
                nm = f"{ch.name}{s}"
                tc_ = actp.tile([128, 256], F32, tag="tc" + ch.name,
                                name="tc" + nm)
                nc.scalar.activation(tc_[:], c_new[:], TANH)
                return tc_

            def emit_h(ch, so, tc_, c_new):
                if real:
                    lp = t_rel if ch.name == "f" else CHUNK - 1 - t_rel
                else:
                    lp = CHUNK + (s & 1)
                dst = tuple(ch.X[k][:, lp * 128:(lp + 1) * 128] for k in (0, 1))
                nc.vector.tensor_mul(dst[0], so[:, 0:128], tc_[:, 0:128])
                nc.vector.tensor_mul(dst[1], so[:, 128:256], tc_[:, 128:256])
                # second warmer anchored on tc
                if WARMERS and s + 1 < K_STEPS:
                    nc.tensor.matmul(gates_next[ch.name][1][0:64, 0:128],
                                     zstat[:], tc_[:, 0:128],
                                     start=False, stop=False,
                                     skip_group_check=True)
                ch.src = dst
                ch.c_prev = c_new
                if s + 1 < K_STEPS:
                    ch.gA, ch.gB = gates_next[ch.name]

            chf, chb = (chains if (s // 2) % 2 == 0 else (chains[1], chains[0]))
            sf_f = emit_sf(chf)
            tg_f = emit_tog_g(chf)
            to_f = emit_tog_o(chf)
            sf_b = emit_sf(chb)
            so_f, c_f = emit_mid(chf, sf_f, tg_f, to_f)
            tc_f = emit_tc(chf, c_f)
            tg_b = emit_tog_g(chb)
            to_b = emit_tog_o(chb)
            emit_h(chf, so_f, tc_f, c_f)
            so_b, c_b = emit_mid(chb, sf_b, tg_b, to_b)
            tc_b = emit_tc(chb, c_b)
            emit_h(chb, so_b, tc_b, c_b)

            for (p0g, gl) in groups_at.get(s, []):
                emit_group(p0g, gl)



    nc.compile()
    return nc


def _np_lstm_fallback(exp, inputs):
    def sigmoid(z):
        return 1.0 / (1.0 + np.exp(-z))

    def lstm(xs, wih, whh, bih, bhh):
        Bb, L, E = xs.shape
        pre = np.einsum("ble,ge->blg", xs, wih) + bih + bhh
        h = np.zeros((Bb, HID), np.float32)
        c = np.zeros((Bb, HID), np.float32)
        hs = np.zeros((Bb, L, HID), np.float32)
        for t in range(L):
            gg = pre[:, t] + h @ whh.T
            i, f, g_, o = np.split(gg, 4, axis=-1)
            c = sigmoid(f) * c + sigmoid(i) * np.tanh(g_)
            h = sigmoid(o) * np.tanh(c)
            hs[:, t] = h
        return hs

    out_f = lstm(exp, inputs["wih_f"], inputs["whh_f"], inputs["bih_f"],
                 inputs["bhh_f"])
    out_b = lstm(exp[:, ::-1], inputs["wih_b"], inputs["whh_b"],
                 inputs["bih_b"], inputs["bhh_b"])[:, ::-1]
    out = np.concatenate([out_f, out_b], axis=-1)
    return out @ inputs["lin_w"].T + inputs["lin_b"]


def make_in_maps(expP, expR, inputs):
    import ml_dtypes
    bf16 = ml_dtypes.bfloat16
    rows, scale = _mchunk_rows()

    def stat_tiles(w):
        # sbuf layout [128, ntiles*128]: tile (m,k) at cols (nk*m+k)*128
        wp = (w.astype(np.float32)[rows] * scale[:, None])
        nk = w.shape[1] // 128
        out = np.zeros((128, 8 * nk * 128), np.float32)
        for m in range(8):
            for k in range(nk):
                out[:, (m * nk + k) * 128:(m * nk + k + 1) * 128] = \
                    wp[m * 128:(m + 1) * 128, k * 128:(k + 1) * 128].T
        return np.ascontiguousarray(out).astype(bf16)

    whhT_f = stat_tiles(inputs["whh_f"])
    whhT_b = stat_tiles(inputs["whh_b"])
    wihT_f = stat_tiles(inputs["wih_f"])
    wihT_b = stat_tiles(inputs["wih_b"])
    lw = inputs["lin_w"].astype(np.float32)
    linT = np.concatenate([np.ascontiguousarray(lw[:, k * 128:(k + 1) * 128].T)
                           for k in range(4)], axis=1).astype(bf16)
    lin_b2 = np.ascontiguousarray(inputs["lin_b"].astype(np.float32)[:, None])

    in_maps = []
    for j in range(N_CORES):
        xein = np.zeros((K_STEPS, EMB, 256), np.float32)
        starts = [2 * j * CHUNK - W,
                  (2 * j + 1) * CHUNK - W,
                  (15 - 2 * j) * CHUNK - W,
                  (14 - 2 * j) * CHUNK - W]
        srcs = [expP, expP, expR, expR]
        for s in range(K_STEPS):
            for ci, (st, src) in enumerate(zip(starts, srcs)):
                p = st + s
                if 0 <= p < L_PAD:
                    xein[s, :, ci * 64:(ci + 1) * 64] = src[:, p].T
        in_maps.append({
            "xein": xein.astype(bf16),
            "whhT_f": whhT_f, "whhT_b": whhT_b,
            "wihT_f": wihT_f, "wihT_b": wihT_b,
            "linT": linT, "lin_b": lin_b2,
        })
    return in_maps


def kernel(**inputs):
    global _COMPILED
    inputs = {k: np.asarray(v) for k, v in inputs.items()}
    x = inputs["x"].astype(np.int64)
    exp, L = _host_expand(x, inputs["embed"].astype(np.float32),
                          inputs["dp_w"].astype(np.float32),
                          inputs["dp_b"].astype(np.float32))

    bias_mag = max(float(np.abs(inputs[k]).max())
                   for k in ("bih_f", "bhh_f", "bih_b", "bhh_b"))
    if L > L_PAD or bias_mag != 0.0:
        f32in = {k: (v.astype(np.float32) if v.dtype.kind == "f" else v)
                 for k, v in inputs.items()}
        return _np_lstm_fallback(exp, f32in).astype(np.float32)

    expP = np.zeros((B, L_PAD, EMB), np.float32)
    expP[:, :L] = exp
    expR = expP[:, ::-1]

    in_maps = make_in_maps(expP, expR, inputs)

    if _COMPILED is None:
        _COMPILED = _build_kernel()
    nc = _COMPILED

    res = run_bass_kernel_spmd(nc, in_maps, core_ids=list(range(N_CORES)))

    out = np.empty((B, L_PAD, MEL), np.float32)
    for j in range(N_CORES):
        om = res.results[j]["out_mel"]          # [MEL, CHUNK, 2, B]
        blk = om.transpose(3, 2, 1, 0).reshape(B, CHUNK2, MEL)
        out[:, j * CHUNK2:(j + 1) * CHUNK2] = blk
    return np.ascontiguousarray(out[:, :L])


if __name__ == "__main__":
    inputs = dict(np.load("/root/problem/inputs.npz"))
    out = kernel(**inputs)
    ref = np.load("/root/problem/expected.npy")
    diff = np.abs(out - ref)
    print("out", out.shape, "absmax diff", diff.max(),
          "rel", diff.max() / np.abs(ref).max())
